# revision 29
# baseline (speedup 1.0000x reference)
"""Trainium2 Bass kernel for the per-node adaptive output layer (gnn_message_passing).

Computation (per node n):
    w1[n] = sum_c label[n,c] * pool1[c]          (64x32)
    w2[n] = sum_c label[n,c] * pool2[c]          (32x12)
    h     = relu(x[:, n, :]) @ w1[n]             (192x64 @ 64x32)
    out   = relu(h) @ w2[n]                      (192x32 @ 32x12)

Distribution: shard N=2048 nodes across 8 NeuronCores (256 nodes/core), weight
pools + labels replicated (labels sharded with N). No collectives needed.

On-device layout (per core, 256 nodes processed in 16 groups of 16 nodes):
  - x DMA'd with fp32->bf16 cast (SWDGE) into [128, 8*192] tiles:
      partition = 64*(m%2) + d, free col = (m//2)*192 + bt   (m = node-in-group)
  - L1 matmuls packed 8-way into the PE array (64x32 tiling mode),
    L2 matmuls packed 16-way (32x32 mode). bf16 inputs, fp32 PSUM.
  - per-node weights computed on device by small K=8 matmuls from the pools.
"""

import sys
import types

import numpy as np

import concourse.bass as bass
import concourse.mybir as mybir
from concourse import tile
from concourse.bass_utils import run_bass_kernel_spmd


def _ensure_ntff_hook():
    """Register the NTFF profiling hook if the image's antenv lacks it.

    bass_utils' axon trace path imports antenv.axon_hooks unconditionally
    when BASS_TRACE is set; provide it from trn_agent_boot when missing so
    tracing works instead of crashing. Best-effort only.
    """
    try:
        from antenv import axon_hooks  # noqa: F401
        return
    except ImportError:
        pass
    try:
        import antenv
        from trn_agent_boot.trn_boot import _ntff_profile_via_ctypes
        hook = [_ntff_profile_via_ctypes("/opt/axon/libaxon_pjrt.so")]
        mod = types.ModuleType("antenv.axon_hooks")
        mod.get_axon_ntff_profile_hook = lambda: hook[0]
        mod.set_axon_ntff_profile_hook = lambda h: hook.__setitem__(0, h)
        sys.modules["antenv.axon_hooks"] = mod
        antenv.axon_hooks = mod
    except Exception:
        pass


_ensure_ntff_hook()

# Problem shape (hardcoded per harness contract)
B, N, T, D = 16, 2048, 12, 64
C, H, O = 8, 32, 12
NCORES = 8
NSH = N // NCORES            # 256 nodes per core
BT = B * T                   # 192
NGROUPS = 16                 # node groups per core
GN = 16                      # nodes per group
NPAIR = NSH // 2             # 128 node pairs per core (w1sb q index)

FP32 = mybir.dt.float32
BF16 = mybir.dt.bfloat16
RELU = mybir.ActivationFunctionType.Relu

# m = index of node within its group (0..15)
#   p  = m % 2          partition half for L1 (0 -> partitions 0:64, 1 -> 64:128)
#   k8 = m // 2         pair index within group (x free-col block, L1 psum slot)
#   r  = k8 % 4         L2 row group (h1 partition group)
#   u  = p + 2*(k8//4)  L2 output column group / w2 idx sub-index


def _m_of(r, u):
    # inverse map: (r, u) -> m
    p = u % 2
    k8 = r + 4 * (u // 2)
    return 2 * k8 + p


last_exec_time_ns = None
last_results = None
_cached_nc = None


def _build_nc(legalize=True, sim_init=False):
    nc = bass.Bass()

    # x packed as 8 blocks of 2 groups: [sb2, 64p+d, g2*1536 + k8*192 + bt]
    x_ext = nc.declare_dram_parameter(
        "x_dev", [NGROUPS // 2, 128, 2 * 8 * BT], FP32, isOutput=False)
    # pools + labels merged into one small param: cols =
    # pool1 (c,h,d) [0:2048] | pool2 (c,o,k) [2048:2432] |
    # label_w1 [2432:2688] | label_w2 [2688:2944]
    wc_ext = nc.declare_dram_parameter("wconst", [C, 2944], FP32, isOutput=False)
    # out layout: [sg, u, o, r*384 + gg*192 + bt]  (sg = g//2, gg = g%2)
    out_ext = nc.declare_dram_parameter(
        "out_dev", [NGROUPS // 2, 4, O, 4 * 2 * BT], FP32, isOutput=True)

    with tile.TileContext(nc) as tc:
        with tc.tile_pool(name="persist", bufs=1) as persist:
            # per-node weights, bf16, matmul-stationary layout
            w1sb = persist.tile([128, NPAIR * H], BF16)       # [64p+d, q*32+h]
            w2sb = persist.tile([128, (NSH // 4) * O], BF16)  # [32r+k, idx*12+o]
            wconst32 = persist.tile([C, 2944], FP32)
            wconst = persist.tile([C, 2944], BF16)

            # consts: fast HWDGE load (doesn't queue behind the big SWDGE x
            # stream), then cast on DVE
            nc.sync.dma_start(wconst32[:], wc_ext[:])
            nc.vector.tensor_copy(wconst[:], wconst32[:])
            pool1 = wconst[:, 0:2048]                # (c, h*64+d)
            pool2 = wconst[:, 2048:2432]             # (c, o*32+k)
            label1 = wconst[:, 2432:2688]            # cols p*128+q
            label2 = wconst[:, 2688:2944]            # cols r*64+idx

            # ---------- hypernetwork: per-node weights ----------
            with tc.tile_pool(name="wpsum", bufs=2, space="PSUM") as wpsum:
                # w1sb: for each h, both parities: out[d, q] = sum_c pool1[c,h,d]*label1[c,q]
                for hc in range(H // 4):         # 8 chunks of 4 h values
                    wp = wpsum.tile([128, 512], FP32, tag="wp")
                    for h4 in range(4):
                        h = hc * 4 + h4
                        for p in range(2):
                            nc.tensor.matmul(
                                wp[64 * p:64 * p + 64, h4 * 128:(h4 + 1) * 128],
                                pool1[:, h * D:(h + 1) * D],            # [8, 64]
                                label1[:, p * NPAIR:(p + 1) * NPAIR],   # [8, 128]
                                tile_position=(0, 64 * p),
                            )
                    # permuted copy psum[p, (h4 q)] -> w1sb[p, q*32 + hc*4 + h4]
                    src = wp[:].rearrange("p (h q) -> p q h", h=4)
                    dst = w1sb[:].rearrange("p (q h) -> p q h", h=H)[
                        :, :, hc * 4:(hc + 1) * 4]
                    if hc % 2 == 0:
                        nc.vector.tensor_copy(dst, src)
                    else:
                        nc.scalar.copy(dst, src)

                # w2sb: out[k, idx] = sum_c pool2[c,o,k]*label2[c, r*64+idx]
                for half in range(2):
                    wp2 = wpsum.tile([128, 384], FP32, tag="wp")
                    for o6 in range(6):
                        o = half * 6 + o6
                        for r in range(4):
                            nc.tensor.matmul(
                                wp2[32 * r:32 * r + 32, o6 * 64:(o6 + 1) * 64],
                                pool2[:, o * H:(o + 1) * H],            # [8, 32]
                                label2[:, r * 64:(r + 1) * 64],         # [8, 64]
                                tile_position=(0, 32 * r),
                            )
                    src = wp2[:].rearrange("p (o i) -> p i o", o=6)
                    dst = w2sb[:].rearrange("p (i o) -> p i o", o=O)[
                        :, :, half * 6:(half + 1) * 6]
                    nc.vector.tensor_copy(dst, src)

            # ---------- main loop over 16-node groups ----------
            with (
                tc.tile_pool(name="xin", bufs=8) as xin,
                tc.tile_pool(name="h1p", bufs=4) as h1p,
                tc.tile_pool(name="outp", bufs=4) as outp,
                tc.tile_pool(name="l1ps", bufs=4, space="PSUM") as l1ps,
                tc.tile_pool(name="l2ps", bufs=4, space="PSUM") as l2ps,
            ):
                l2banks = None
                xt2 = None
                for g in range(NGROUPS):
                    if g % 2 == 0:
                        xt2 = xin.tile([128, 2 * 8 * BT], BF16, tag="x")
                        nc.gpsimd.dma_start(xt2[:], x_ext[g // 2])  # cast f32->bf16
                        nc.vector.tensor_scalar_max(xt2[:], xt2[:], 0.0)  # relu(x)
                    xt = xt2[:, (g % 2) * 8 * BT:(g % 2 + 1) * 8 * BT]

                    # layer 1: 16 matmuls, 8-way PE tiling (64x32)
                    pA = l1ps.tile([128, 384], FP32, tag="l1")  # even (p=0) nodes
                    pB = l1ps.tile([128, 384], FP32, tag="l1")  # odd  (p=1) nodes
                    for m in range(GN):
                        p, k8 = m % 2, m // 2
                        q = g * 8 + k8
                        j, off = k8 % 4, BT * (k8 // 4)
                        dst = pA if p == 0 else pB
                        nc.tensor.matmul(
                            dst[32 * j:32 * j + 32, off:off + BT],
                            w1sb[64 * p:64 * p + 64, q * H:(q + 1) * H],
                            xt[64 * p:64 * p + 64, k8 * BT:(k8 + 1) * BT],
                            tile_position=(64 * p, 32 * j),
                        )

                    # relu + cast to bf16, psum -> sbuf
                    h1A = h1p.tile([128, 384], BF16, tag="h1")
                    h1B = h1p.tile([128, 384], BF16, tag="h1")
                    nc.scalar.activation(h1A[:], pA[:], RELU)
                    nc.vector.tensor_scalar_max(h1B[:], pB[:], 0.0)

                    # layer 2: 16 matmuls, 16-way PE tiling (32x32)
                    if g % 2 == 0:
                        l2banks = [
                            l2ps.tile([128, 384], FP32, tag="l2", name=f"l2b{r}")
                            for r in range(4)]
                        if sim_init:
                            # CoreSim-only: matmuls leave 20 of each 32
                            # partitions unwritten; the full-tile evacuation
                            # copy reads them (harmless on HW, flagged in sim)
                            for bank in l2banks:
                                nc.vector.memset(bank[:], 0.0)
                    for m in range(GN):
                        p, k8 = m % 2, m // 2
                        r, u = k8 % 4, (m % 2) + 2 * (k8 // 4)
                        idx = g * 4 + u
                        src = h1A if p == 0 else h1B
                        nc.tensor.matmul(
                            l2banks[r][32 * u:32 * u + O,
                                       BT * (g % 2):BT * (g % 2) + BT],
                            w2sb[32 * r:32 * r + 32, idx * O:(idx + 1) * O],
                            src[32 * r:32 * r + 32,
                                BT * (k8 // 4):BT * (k8 // 4) + BT],
                            tile_position=(32 * r, 32 * u),
                        )

                    # every 2 groups: evacuate psum and DMA out
                    if g % 2 == 1:
                        sg = g // 2
                        otq = outp.tile([128, 4 * 384], FP32, tag="out")
                        for r in range(4):
                            if r == 0:
                                nc.vector.tensor_copy(
                                    otq[:, r * 384:(r + 1) * 384], l2banks[r][:])
                            else:
                                nc.scalar.copy(
                                    otq[:, r * 384:(r + 1) * 384], l2banks[r][:])
                        for u in range(4):
                            eng = nc.sync if u % 2 == 0 else nc.scalar
                            eng.dma_start(out_ext[sg, u],
                                          otq[32 * u:32 * u + O, :])

    nc.finalize()
    if legalize:
        _legalize_waits(nc)
    return nc


def _legalize_waits(nc, keep_max=1, nop_max=1):
    """Hoist excess per-instruction semaphore waits onto same-engine NOPs.

    This walrus build rejects instructions carrying more than a couple of
    sync-wait commands ("Too many sync wait commands"). Tile attaches all
    required waits directly to consumer instructions; split them onto
    preceding InstNoOps on the same engine (semantically identical: the
    sequencer performs the waits in order before the real instruction).
    """
    ctr = [0]

    def mknop(engine, waits):
        ctr[0] += 1
        return mybir.InstNoOp(
            name=f"I-whoist-{ctr[0]}", engine=engine, bass_nofuse=True,
            sync_info=mybir.SyncInfo(on_wait=list(waits), on_update=[]))

    for f in nc.m.functions:
        for blk in f.blocks:
            out = []
            for inst in blk.instructions:
                si = getattr(inst, 'sync_info', None)
                eng = getattr(inst, 'engine', None)
                if si is not None and eng is not None and len(si.on_wait) > keep_max:
                    waits = list(si.on_wait)
                    keep, hoist = waits[:keep_max], waits[keep_max:]
                    for i in range(0, len(hoist), nop_max):
                        out.append(mknop(eng, hoist[i:i + nop_max]))
                    inst.sync_info = mybir.SyncInfo(
                        on_wait=keep, on_update=list(si.on_update))
                out.append(inst)
            blk.instructions = out


def _get_nc():
    global _cached_nc
    if _cached_nc is None:
        _cached_nc = _build_nc()
    return _cached_nc


def _prep_inputs(x, node_label, weights_pool1, weights_pool2):
    """Shard + pre-transpose full inputs into per-core in_maps."""
    x = np.ascontiguousarray(x, dtype=np.float32)
    node_label = np.ascontiguousarray(node_label, dtype=np.float32)
    p1 = np.ascontiguousarray(
        weights_pool1.transpose(0, 2, 1), dtype=np.float32).reshape(C, H * D)
    p2 = np.ascontiguousarray(
        weights_pool2.transpose(0, 2, 1), dtype=np.float32).reshape(C, O * H)

    # x -> [n, d, bt]
    x_t = np.ascontiguousarray(x.transpose(1, 3, 0, 2)).reshape(N, D, BT)

    # m index table for (r, u)
    m_arr = np.empty((4, 4), dtype=np.int64)
    for r in range(4):
        for u in range(4):
            m_arr[r, u] = _m_of(r, u)

    in_maps = []
    for k in range(NCORES):
        lab = node_label[k * NSH:(k + 1) * NSH]            # [256, 8]
        xs = x_t[k * NSH:(k + 1) * NSH]                    # [256, 64, 192]
        # x_dev[g, 64p+d, k8*192+bt] = x_t[16g + 2*k8 + p, d, bt]
        xdev = xs.reshape(NGROUPS, 8, 2, D, BT).transpose(0, 2, 3, 1, 4)
        xdev = xdev.reshape(NGROUPS, 128, 8 * BT)
        # pack 2 groups per DMA block: [sb2, part, g2*1536 + c]
        xdev = np.ascontiguousarray(
            xdev.reshape(8, 2, 128, 8 * BT).transpose(0, 2, 1, 3)
        ).reshape(8, 128, 2 * 8 * BT)
        # label_w1[c, p*128+q] = lab[2q+p, c]
        lw1 = lab.reshape(NPAIR, 2, C).transpose(2, 1, 0).reshape(C, NSH)
        # label_w2[c, r*64 + 4g + u] = lab[16g + m_arr[r,u], c]
        gidx = (16 * np.arange(NGROUPS))[None, :, None] + m_arr[:, None, :]
        lw2 = lab[gidx.reshape(-1)].reshape(4, NGROUPS, 4, C) \
            .transpose(3, 0, 1, 2).reshape(C, NSH)
        wconst = np.ascontiguousarray(
            np.concatenate([p1, p2, lw1, lw2], axis=1))    # [8, 2944]
        in_maps.append({"x_dev": xdev, "wconst": wconst})
    return in_maps


def _unpack_outputs(results):
    """Per-core out_dev [sg, r, u, o, gg, bt] -> full (B, N, T, O)."""
    m_arr = np.empty((4, 4), dtype=np.int64)
    for r in range(4):
        for u in range(4):
            m_arr[r, u] = _m_of(r, u)

    out = np.empty((B, N, T, O), dtype=np.float32)
    for k in range(NCORES):
        od = np.asarray(results[k]["out_dev"]).reshape(
            NGROUPS // 2, 4, O, 4, 2, BT)       # [sg, u, o, r, gg, bt]
        od = od.transpose(0, 4, 3, 1, 2, 5)     # [sg, gg, r, u, o, bt]
        # node local index l = 16*(2*sg+gg) + m_arr[r, u]
        sg = np.arange(NGROUPS // 2)[:, None, None, None]
        gg = np.arange(2)[None, :, None, None]
        l_arr = 16 * (2 * sg + gg) + m_arr[None, None, :, :]
        out_core = np.empty((NSH, O, BT), dtype=np.float32)
        out_core[l_arr.reshape(-1)] = od.reshape(-1, O, BT)
        # out[b, n, t, o] = out_core[nl, o, b*T+t]
        oc = out_core.reshape(NSH, O, B, T).transpose(2, 0, 3, 1)
        out[:, k * NSH:(k + 1) * NSH] = oc
    return out


def kernel(x, node_label, weights_pool1, weights_pool2):
    global last_exec_time_ns, last_results
    nc = _get_nc()
    in_maps = _prep_inputs(x, node_label, weights_pool1, weights_pool2)
    res = run_bass_kernel_spmd(nc, in_maps, core_ids=list(range(NCORES)))
    last_exec_time_ns = res.exec_time_ns
    last_results = res
    return _unpack_outputs(res.results)


# revision 31
# speedup vs baseline: 1.1837x; 1.1837x over previous
"""Trainium2 Bass kernel for the per-node adaptive output layer (gnn_message_passing).

Computation (per node n):
    w1[n] = sum_c label[n,c] * pool1[c]          (64x32)
    w2[n] = sum_c label[n,c] * pool2[c]          (32x12)
    h     = relu(x[:, n, :]) @ w1[n]             (192x64 @ 64x32)
    out   = relu(h) @ w2[n]                      (192x32 @ 32x12)

Distribution: shard N=2048 nodes across 8 NeuronCores (256 nodes/core), weight
pools + labels replicated (labels sharded with N). No collectives needed.

On-device layout (per core, 256 nodes processed in 16 groups of 16 nodes):
  - x DMA'd with fp32->bf16 cast (SWDGE) into [128, 8*192] tiles:
      partition = 64*(m%2) + d, free col = (m//2)*192 + bt   (m = node-in-group)
  - L1 matmuls packed 8-way into the PE array (64x32 tiling mode),
    L2 matmuls packed 16-way (32x32 mode). bf16 inputs, fp32 PSUM.
  - per-node weights computed on device by small K=8 matmuls from the pools.
"""

import sys
import types

import numpy as np

import concourse.bass as bass
import concourse.mybir as mybir
from concourse import tile
from concourse.bass_utils import run_bass_kernel_spmd


def _ensure_ntff_hook():
    """Register the NTFF profiling hook if the image's antenv lacks it.

    bass_utils' axon trace path imports antenv.axon_hooks unconditionally
    when BASS_TRACE is set; provide it from trn_agent_boot when missing so
    tracing works instead of crashing. Best-effort only.
    """
    try:
        from antenv import axon_hooks  # noqa: F401
        return
    except ImportError:
        pass
    try:
        import antenv
        from trn_agent_boot.trn_boot import _ntff_profile_via_ctypes
        hook = [_ntff_profile_via_ctypes("/opt/axon/libaxon_pjrt.so")]
        mod = types.ModuleType("antenv.axon_hooks")
        mod.get_axon_ntff_profile_hook = lambda: hook[0]
        mod.set_axon_ntff_profile_hook = lambda h: hook.__setitem__(0, h)
        sys.modules["antenv.axon_hooks"] = mod
        antenv.axon_hooks = mod
    except Exception:
        pass


_ensure_ntff_hook()

# Problem shape (hardcoded per harness contract)
B, N, T, D = 16, 2048, 12, 64
C, H, O = 8, 32, 12
NCORES = 8
NSH = N // NCORES            # 256 nodes per core
BT = B * T                   # 192
NGROUPS = 16                 # node groups per core
GN = 16                      # nodes per group
NPAIR = NSH // 2             # 128 node pairs per core (w1sb q index)

FP32 = mybir.dt.float32
BF16 = mybir.dt.bfloat16
RELU = mybir.ActivationFunctionType.Relu

# m = index of node within its group (0..15)
#   p  = m % 2          partition half for L1 (0 -> partitions 0:64, 1 -> 64:128)
#   k8 = m // 2         pair index within group (x free-col block, L1 psum slot)
#   r  = k8 % 4         L2 row group (h1 partition group)
#   u  = p + 2*(k8//4)  L2 output column group / w2 idx sub-index


def _m_of(r, u):
    # inverse map: (r, u) -> m
    p = u % 2
    k8 = r + 4 * (u // 2)
    return 2 * k8 + p


last_exec_time_ns = None
last_results = None
_cached_nc = None


def _build_nc(legalize=True, sim_init=False):
    nc = bass.Bass()

    # x packed as 8 blocks of 2 groups: [sb2, 64p+d, g2*1536 + k8*192 + bt]
    x_ext = nc.declare_dram_parameter(
        "x_dev", [NGROUPS // 2, 128, 2 * 8 * BT], FP32, isOutput=False)
    # pools + labels merged into one small param: cols =
    # pool1 (c,h,d) [0:2048] | pool2 (c,o,k) [2048:2432] |
    # label_w1 [2432:2688] | label_w2 [2688:2944]
    wc_ext = nc.declare_dram_parameter("wconst", [C, 2944], FP32, isOutput=False)
    # out layout: [sg, u, o, r*384 + gg*192 + bt]  (sg = g//2, gg = g%2)
    out_ext = nc.declare_dram_parameter(
        "out_dev", [NGROUPS // 2, 4, O, 4 * 2 * BT], FP32, isOutput=True)

    with tile.TileContext(nc) as tc:
        with tc.tile_pool(name="persist", bufs=1) as persist:
            # per-node weights, bf16, matmul-stationary layout
            w1sb = persist.tile([128, NPAIR * H], BF16)       # [64p+d, q*32+h]
            w2sb = persist.tile([128, (NSH // 4) * O], BF16)  # [32r+k, idx*12+o]
            wconst = persist.tile([C, 2944], BF16)

            # fp32 -> bf16 cast during DMA (SWDGE), first in the queue
            nc.gpsimd.dma_start(wconst[:], wc_ext[:])
            pool1 = wconst[:, 0:2048]                # (c, h*64+d)
            pool2 = wconst[:, 2048:2432]             # (c, o*32+k)
            label1 = wconst[:, 2432:2688]            # cols p*128+q
            label2 = wconst[:, 2688:2944]            # cols r*64+idx

            # ---------- hypernetwork: per-node weights ----------
            with tc.tile_pool(name="wpsum", bufs=2, space="PSUM") as wpsum:
                # w1sb: for each h, both parities: out[d, q] = sum_c pool1[c,h,d]*label1[c,q]
                for hc in range(H // 4):         # 8 chunks of 4 h values
                    wp = wpsum.tile([128, 512], FP32, tag="wp")
                    for h4 in range(4):
                        h = hc * 4 + h4
                        for p in range(2):
                            nc.tensor.matmul(
                                wp[64 * p:64 * p + 64, h4 * 128:(h4 + 1) * 128],
                                pool1[:, h * D:(h + 1) * D],            # [8, 64]
                                label1[:, p * NPAIR:(p + 1) * NPAIR],   # [8, 128]
                                tile_position=(0, 64 * p),
                            )
                    # permuted copy psum[p, (h4 q)] -> w1sb[p, q*32 + hc*4 + h4]
                    src = wp[:].rearrange("p (h q) -> p q h", h=4)
                    dst = w1sb[:].rearrange("p (q h) -> p q h", h=H)[
                        :, :, hc * 4:(hc + 1) * 4]
                    if hc % 2 == 0:
                        nc.vector.tensor_copy(dst, src)
                    else:
                        nc.scalar.copy(dst, src)

                # w2sb: out[k, idx] = sum_c pool2[c,o,k]*label2[c, r*64+idx]
                for half in range(2):
                    wp2 = wpsum.tile([128, 384], FP32, tag="wp")
                    for o6 in range(6):
                        o = half * 6 + o6
                        for r in range(4):
                            nc.tensor.matmul(
                                wp2[32 * r:32 * r + 32, o6 * 64:(o6 + 1) * 64],
                                pool2[:, o * H:(o + 1) * H],            # [8, 32]
                                label2[:, r * 64:(r + 1) * 64],         # [8, 64]
                                tile_position=(0, 32 * r),
                            )
                    src = wp2[:].rearrange("p (o i) -> p i o", o=6)
                    dst = w2sb[:].rearrange("p (i o) -> p i o", o=O)[
                        :, :, half * 6:(half + 1) * 6]
                    nc.vector.tensor_copy(dst, src)

            # ---------- main loop over 16-node groups ----------
            with (
                tc.tile_pool(name="xin", bufs=8) as xin,
                tc.tile_pool(name="h1p", bufs=4) as h1p,
                tc.tile_pool(name="outp", bufs=4) as outp,
                tc.tile_pool(name="l1ps", bufs=4, space="PSUM") as l1ps,
                tc.tile_pool(name="l2ps", bufs=4, space="PSUM") as l2ps,
            ):
                l2banks = None
                xt2 = None
                h1s = {}
                # software pipeline with 1-group skew: issue L1(g) before
                # L2(g-1) so the PE (in-order) works on the next group's
                # layer 1 while ACT/DVE evacuate the previous group's h1.
                for g in range(NGROUPS + 1):
                    if g < NGROUPS:
                        if g % 2 == 0:
                            xt2 = xin.tile([128, 2 * 8 * BT], BF16, tag="x")
                            nc.gpsimd.dma_start(xt2[:], x_ext[g // 2])  # cast
                            nc.vector.tensor_scalar_max(
                                xt2[:], xt2[:], 0.0)  # relu(x)
                        xt = xt2[:, (g % 2) * 8 * BT:(g % 2 + 1) * 8 * BT]

                        # layer 1: 16 matmuls, 8-way PE tiling (64x32)
                        pA = l1ps.tile([128, 384], FP32, tag="l1")  # p=0 nodes
                        pB = l1ps.tile([128, 384], FP32, tag="l1")  # p=1 nodes
                        for m in range(GN):
                            p, k8 = m % 2, m // 2
                            q = g * 8 + k8
                            j, off = k8 % 4, BT * (k8 // 4)
                            dst = pA if p == 0 else pB
                            nc.tensor.matmul(
                                dst[32 * j:32 * j + 32, off:off + BT],
                                w1sb[64 * p:64 * p + 64, q * H:(q + 1) * H],
                                xt[64 * p:64 * p + 64, k8 * BT:(k8 + 1) * BT],
                                tile_position=(64 * p, 32 * j),
                            )

                        # relu + cast to bf16, psum -> sbuf
                        h1A = h1p.tile([128, 384], BF16, tag="h1")
                        h1B = h1p.tile([128, 384], BF16, tag="h1")
                        nc.scalar.activation(h1A[:], pA[:], RELU)
                        nc.vector.tensor_scalar_max(h1B[:], pB[:], 0.0)
                        h1s[g] = (h1A, h1B)

                    if g < 1:
                        continue
                    gg = g - 1      # layer-2 stage processes group g-1
                    h1A, h1B = h1s.pop(gg)

                    # layer 2: 16 matmuls, 16-way PE tiling (32x32)
                    if gg % 2 == 0:
                        l2banks = [
                            l2ps.tile([128, 384], FP32, tag="l2", name=f"l2b{r}")
                            for r in range(4)]
                        if sim_init:
                            # CoreSim-only: matmuls leave 20 of each 32
                            # partitions unwritten; the full-tile evacuation
                            # copy reads them (harmless on HW, flagged in sim)
                            for bank in l2banks:
                                nc.vector.memset(bank[:], 0.0)
                    for m in range(GN):
                        p, k8 = m % 2, m // 2
                        r, u = k8 % 4, (m % 2) + 2 * (k8 // 4)
                        idx = gg * 4 + u
                        src = h1A if p == 0 else h1B
                        nc.tensor.matmul(
                            l2banks[r][32 * u:32 * u + O,
                                       BT * (gg % 2):BT * (gg % 2) + BT],
                            w2sb[32 * r:32 * r + 32, idx * O:(idx + 1) * O],
                            src[32 * r:32 * r + 32,
                                BT * (k8 // 4):BT * (k8 // 4) + BT],
                            tile_position=(32 * r, 32 * u),
                        )

                    # every 2 groups: evacuate psum and DMA out
                    if gg % 2 == 1:
                        sg = gg // 2
                        otq = outp.tile([128, 4 * 384], FP32, tag="out")
                        for r in range(4):
                            if r == 0:
                                nc.vector.tensor_copy(
                                    otq[:, r * 384:(r + 1) * 384], l2banks[r][:])
                            else:
                                nc.scalar.copy(
                                    otq[:, r * 384:(r + 1) * 384], l2banks[r][:])
                        for u in range(4):
                            eng = nc.sync if u % 2 == 0 else nc.scalar
                            eng.dma_start(out_ext[sg, u],
                                          otq[32 * u:32 * u + O, :])

    nc.finalize()
    if legalize:
        _legalize_waits(nc)
    return nc


def _legalize_waits(nc, keep_max=1, nop_max=1):
    """Hoist excess per-instruction semaphore waits onto same-engine NOPs.

    This walrus build rejects instructions carrying more than a couple of
    sync-wait commands ("Too many sync wait commands"). Tile attaches all
    required waits directly to consumer instructions; split them onto
    preceding InstNoOps on the same engine (semantically identical: the
    sequencer performs the waits in order before the real instruction).
    """
    ctr = [0]

    def mknop(engine, waits):
        ctr[0] += 1
        return mybir.InstNoOp(
            name=f"I-whoist-{ctr[0]}", engine=engine, bass_nofuse=True,
            sync_info=mybir.SyncInfo(on_wait=list(waits), on_update=[]))

    for f in nc.m.functions:
        for blk in f.blocks:
            out = []
            for inst in blk.instructions:
                si = getattr(inst, 'sync_info', None)
                eng = getattr(inst, 'engine', None)
                if si is not None and eng is not None and len(si.on_wait) > keep_max:
                    waits = list(si.on_wait)
                    keep, hoist = waits[:keep_max], waits[keep_max:]
                    for i in range(0, len(hoist), nop_max):
                        out.append(mknop(eng, hoist[i:i + nop_max]))
                    inst.sync_info = mybir.SyncInfo(
                        on_wait=keep, on_update=list(si.on_update))
                out.append(inst)
            blk.instructions = out


def _get_nc():
    global _cached_nc
    if _cached_nc is None:
        _cached_nc = _build_nc()
    return _cached_nc


def _prep_inputs(x, node_label, weights_pool1, weights_pool2):
    """Shard + pre-transpose full inputs into per-core in_maps."""
    x = np.ascontiguousarray(x, dtype=np.float32)
    node_label = np.ascontiguousarray(node_label, dtype=np.float32)
    p1 = np.ascontiguousarray(
        weights_pool1.transpose(0, 2, 1), dtype=np.float32).reshape(C, H * D)
    p2 = np.ascontiguousarray(
        weights_pool2.transpose(0, 2, 1), dtype=np.float32).reshape(C, O * H)

    # x -> [n, d, bt]
    x_t = np.ascontiguousarray(x.transpose(1, 3, 0, 2)).reshape(N, D, BT)

    # m index table for (r, u)
    m_arr = np.empty((4, 4), dtype=np.int64)
    for r in range(4):
        for u in range(4):
            m_arr[r, u] = _m_of(r, u)

    in_maps = []
    for k in range(NCORES):
        lab = node_label[k * NSH:(k + 1) * NSH]            # [256, 8]
        xs = x_t[k * NSH:(k + 1) * NSH]                    # [256, 64, 192]
        # x_dev[g, 64p+d, k8*192+bt] = x_t[16g + 2*k8 + p, d, bt]
        xdev = xs.reshape(NGROUPS, 8, 2, D, BT).transpose(0, 2, 3, 1, 4)
        xdev = xdev.reshape(NGROUPS, 128, 8 * BT)
        # pack 2 groups per DMA block: [sb2, part, g2*1536 + c]
        xdev = np.ascontiguousarray(
            xdev.reshape(8, 2, 128, 8 * BT).transpose(0, 2, 1, 3)
        ).reshape(8, 128, 2 * 8 * BT)
        # label_w1[c, p*128+q] = lab[2q+p, c]
        lw1 = lab.reshape(NPAIR, 2, C).transpose(2, 1, 0).reshape(C, NSH)
        # label_w2[c, r*64 + 4g + u] = lab[16g + m_arr[r,u], c]
        gidx = (16 * np.arange(NGROUPS))[None, :, None] + m_arr[:, None, :]
        lw2 = lab[gidx.reshape(-1)].reshape(4, NGROUPS, 4, C) \
            .transpose(3, 0, 1, 2).reshape(C, NSH)
        wconst = np.ascontiguousarray(
            np.concatenate([p1, p2, lw1, lw2], axis=1))    # [8, 2944]
        in_maps.append({"x_dev": xdev, "wconst": wconst})
    return in_maps


def _unpack_outputs(results):
    """Per-core out_dev [sg, r, u, o, gg, bt] -> full (B, N, T, O)."""
    m_arr = np.empty((4, 4), dtype=np.int64)
    for r in range(4):
        for u in range(4):
            m_arr[r, u] = _m_of(r, u)

    out = np.empty((B, N, T, O), dtype=np.float32)
    for k in range(NCORES):
        od = np.asarray(results[k]["out_dev"]).reshape(
            NGROUPS // 2, 4, O, 4, 2, BT)       # [sg, u, o, r, gg, bt]
        od = od.transpose(0, 4, 3, 1, 2, 5)     # [sg, gg, r, u, o, bt]
        # node local index l = 16*(2*sg+gg) + m_arr[r, u]
        sg = np.arange(NGROUPS // 2)[:, None, None, None]
        gg = np.arange(2)[None, :, None, None]
        l_arr = 16 * (2 * sg + gg) + m_arr[None, None, :, :]
        out_core = np.empty((NSH, O, BT), dtype=np.float32)
        out_core[l_arr.reshape(-1)] = od.reshape(-1, O, BT)
        # out[b, n, t, o] = out_core[nl, o, b*T+t]
        oc = out_core.reshape(NSH, O, B, T).transpose(2, 0, 3, 1)
        out[:, k * NSH:(k + 1) * NSH] = oc
    return out


def kernel(x, node_label, weights_pool1, weights_pool2):
    global last_exec_time_ns, last_results
    nc = _get_nc()
    in_maps = _prep_inputs(x, node_label, weights_pool1, weights_pool2)
    res = run_bass_kernel_spmd(nc, in_maps, core_ids=list(range(NCORES)))
    last_exec_time_ns = res.exec_time_ns
    last_results = res
    return _unpack_outputs(res.results)


# revision 34
# speedup vs baseline: 1.5006x; 1.2677x over previous
"""Trainium2 Bass kernel for the per-node adaptive output layer (gnn_message_passing).

Computation (per node n):
    w1[n] = sum_c label[n,c] * pool1[c]          (64x32)
    w2[n] = sum_c label[n,c] * pool2[c]          (32x12)
    h     = relu(x[:, n, :]) @ w1[n]             (192x64 @ 64x32)
    out   = relu(h) @ w2[n]                      (192x32 @ 32x12)

Distribution: shard N=2048 nodes across 8 NeuronCores (256 nodes/core),
weight pools replicated, labels sharded with N. No collectives.

Per-core schedule (256 nodes, 16 groups of 16 nodes = 8 even/odd pairs):
  - x arrives via SWDGE cast-DMA (fp32->bf16) in [128, 2*8*192] blocks:
      partition = 64*(m%2) + d, free col = (m//2)*192 + bt
  - Layer 1 packs an (even, odd) node pair into one K=128 matmul with a
    block-diagonal [128, 64] weight tile (8 MMs/group, 2-way column tiling).
  - Layer 2 packs FOUR nodes into one K=128 matmul with a 4x[32,12]
    block-diagonal weight tile (4 MMs/group); outputs land densely on
    48-partition spans, giving well-formed output DMAs.
  - Per-node weights are computed on device from the pools (K=8 matmuls),
    fp32 PSUM, written to bf16 block-diagonal stationary layouts.
"""

import sys
import types

import numpy as np

import concourse.bass as bass
import concourse.mybir as mybir
from concourse import tile
from concourse.bass_utils import run_bass_kernel_spmd


def _ensure_ntff_hook():
    """Register the NTFF profiling hook if the image's antenv lacks it.

    bass_utils' axon trace path imports antenv.axon_hooks unconditionally
    when BASS_TRACE is set; provide it from trn_agent_boot when missing so
    tracing works instead of crashing. Best-effort only.
    """
    try:
        from antenv import axon_hooks  # noqa: F401
        return
    except ImportError:
        pass
    try:
        import antenv
        from trn_agent_boot.trn_boot import _ntff_profile_via_ctypes
        hook = [_ntff_profile_via_ctypes("/opt/axon/libaxon_pjrt.so")]
        mod = types.ModuleType("antenv.axon_hooks")
        mod.get_axon_ntff_profile_hook = lambda: hook[0]
        mod.set_axon_ntff_profile_hook = lambda h: hook.__setitem__(0, h)
        sys.modules["antenv.axon_hooks"] = mod
        antenv.axon_hooks = mod
    except Exception:
        pass


_ensure_ntff_hook()

# Problem shape (hardcoded per harness contract)
B, N, T, D = 16, 2048, 12, 64
C, H, O = 8, 32, 12
NCORES = 8
NSH = N // NCORES            # 256 nodes per core
BT = B * T                   # 192
NGROUPS = 16                 # node groups per core
GN = 16                      # nodes per group
NPAIR = NSH // 2             # 128 node pairs per core

FP32 = mybir.dt.float32
BF16 = mybir.dt.bfloat16
RELU = mybir.ActivationFunctionType.Relu

# Within a group, node index m (0..15): p = m%2 (L1 partition half),
# k8 = m//2 (pair index / x free-col block).
# Layer-2 regrouping: each L2 matmul j covers 4 nodes, one per slot
# s (0..3); slot s of matmul (yb, cb) is node k8 = 4*yb + 2*cb + s//2,
# p = s%2.  (yb = psum bank X/Y of layer 1, cb = col block within bank.)


def _m_of(yb, cb, s):
    k8 = 4 * yb + 2 * cb + (s // 2)
    return 2 * k8 + (s % 2)


last_exec_time_ns = None
last_results = None
_cached_nc = None


def _build_nc(legalize=True, sim_init=False):
    nc = bass.Bass()

    # x packed as 8 blocks of 2 groups: [sb2, 64p+d, g2*1536 + k8*192 + bt]
    x_ext = nc.declare_dram_parameter(
        "x_dev", [NGROUPS // 2, 128, 2 * 8 * BT], FP32, isOutput=False)
    # pools + labels merged: pool1 (c,h,d) [0:2048] | pool2 (c,o,k)
    # [2048:2432] | label_w1 [2432:2688] | label_w2 [2688:2944]
    wc_ext = nc.declare_dram_parameter("wconst", [C, 2944], FP32, isOutput=False)
    # out: [sg, half, 12s+o, gg*384 + yb*192 + bt]
    out_ext = nc.declare_dram_parameter(
        "out_dev", [NGROUPS // 2, 2, 48, 2 * 2 * BT], FP32, isOutput=True)

    with tile.TileContext(nc) as tc:
        with tc.tile_pool(name="persist", bufs=1) as persist:
            # block-diagonal stationary weights, bf16
            # w1bd[64p+d, q*64 + 32p + h] = w1[2q+p][d, h]; zeros elsewhere
            w1bd = persist.tile([128, NPAIR * 2 * H], BF16)
            # w2bd[32s+k, j*48 + 12s + o] = w2[node(j, s)][k, o]; zeros else
            w2bd = persist.tile([128, (NSH // 4) * 4 * O], BF16)
            wconst = persist.tile([C, 2944], BF16)

            nc.vector.memset(w1bd[:], 0.0)
            nc.vector.memset(w2bd[:], 0.0)

            # fp32 -> bf16 cast during DMA (SWDGE), first in the queue
            nc.gpsimd.dma_start(wconst[:], wc_ext[:])
            pool1 = wconst[:, 0:2048]                # (c, h*64+d)
            pool2 = wconst[:, 2048:2432]             # (c, o*32+k)
            label1 = wconst[:, 2432:2688]            # cols p*128+q
            label2 = wconst[:, 2688:2944]            # cols s*64 + (g*4+j_local)

            # ---------- hypernetwork: per-node weights ----------
            with tc.tile_pool(name="wpsum", bufs=4, space="PSUM") as wpsum:
                # w1: out[d, q] = sum_c pool1[c,h,d]*label1[c,q], both parities
                for hc in range(H // 4):         # 8 chunks of 4 h values
                    wp = wpsum.tile([128, 512], FP32, tag="wp")
                    for h4 in range(4):
                        h = hc * 4 + h4
                        for p in range(2):
                            nc.tensor.matmul(
                                wp[64 * p:64 * p + 64, h4 * 128:(h4 + 1) * 128],
                                pool1[:, h * D:(h + 1) * D],            # [8, 64]
                                label1[:, p * NPAIR:(p + 1) * NPAIR],   # [8, 128]
                                tile_position=(0, 64 * p),
                            )
                    # psum[64p+d, (h4 q)] -> w1bd[64p+d, q*64 + 32p + hc*4+h4]
                    for p in range(2):
                        src = wp[64 * p:64 * p + 64, :].rearrange(
                            "p (h q) -> p q h", h=4)
                        dst = w1bd[64 * p:64 * p + 64, :].rearrange(
                            "p (q h) -> p q h", h=2 * H)[
                            :, :, 32 * p + hc * 4:32 * p + hc * 4 + 4]
                        if (hc * 2 + p) % 2 == 0:
                            nc.vector.tensor_copy(dst, src)
                        else:
                            nc.scalar.copy(dst, src)

                # w2: out[k, idx] = sum_c pool2[c,o,k]*label2[c, s*64+idx]
                for half in range(2):
                    wp2 = wpsum.tile([128, 384], FP32, tag="wp")
                    for o6 in range(6):
                        o = half * 6 + o6
                        for s in range(4):
                            nc.tensor.matmul(
                                wp2[32 * s:32 * s + 32, o6 * 64:(o6 + 1) * 64],
                                pool2[:, o * H:(o + 1) * H],            # [8, 32]
                                label2[:, s * 64:(s + 1) * 64],         # [8, 64]
                                tile_position=(0, 32 * s),
                            )
                    # psum[32s+k, (o6 idx)] -> w2bd[32s+k, idx*48 + 12s + o]
                    for s in range(4):
                        src = wp2[32 * s:32 * s + 32, :].rearrange(
                            "p (o i) -> p i o", o=6)
                        dst = w2bd[32 * s:32 * s + 32, :].rearrange(
                            "p (i o) -> p i o", o=4 * O)[
                            :, :, 12 * s + half * 6:12 * s + half * 6 + 6]
                        if (half * 4 + s) % 2 == 0:
                            nc.vector.tensor_copy(dst, src)
                        else:
                            nc.scalar.copy(dst, src)

            # ---------- main loop ----------
            with (
                tc.tile_pool(name="xin", bufs=8) as xin,
                tc.tile_pool(name="h1p", bufs=4) as h1p,
                tc.tile_pool(name="outp", bufs=4) as outp,
                tc.tile_pool(name="l1ps", bufs=4, space="PSUM") as l1ps,
                tc.tile_pool(name="l2ps", bufs=4, space="PSUM") as l2ps,
            ):
                otq = None
                xt2 = None
                h1s = {}
                # software pipeline, 1-group skew: L1(g) issues before L2(g-1)
                for g in range(NGROUPS + 1):
                    if g < NGROUPS:
                        if g % 2 == 0:
                            xt2 = xin.tile([128, 2 * 8 * BT], BF16, tag="x")
                            nc.gpsimd.dma_start(xt2[:], x_ext[g // 2])  # cast
                            nc.vector.tensor_scalar_max(
                                xt2[:], xt2[:], 0.0)  # relu(x)
                        xt = xt2[:, (g % 2) * 8 * BT:(g % 2 + 1) * 8 * BT]

                        # layer 1: 8 block-diagonal pair matmuls (128x64)
                        pX = l1ps.tile([128, 384], FP32, tag="l1")  # pairs 0-3
                        pY = l1ps.tile([128, 384], FP32, tag="l1")  # pairs 4-7
                        for k8 in range(8):
                            q = g * 8 + k8
                            dst = pX if k8 < 4 else pY
                            cb = (k8 % 4) // 2
                            nc.tensor.matmul(
                                dst[64 * (k8 % 2):64 * (k8 % 2) + 64,
                                    # pairs (0,1)|(2,3) share a col range
                                    192 * cb:192 * cb + BT],
                                w1bd[:, q * 64:(q + 1) * 64],
                                xt[:, k8 * BT:(k8 + 1) * BT],
                                tile_position=(0, 64 * (k8 % 2)),
                            )

                        # relu + cast to bf16, psum -> sbuf
                        h1X = h1p.tile([128, 384], BF16, tag="h1")
                        h1Y = h1p.tile([128, 384], BF16, tag="h1")
                        nc.scalar.activation(h1X[:], pX[:], RELU)
                        nc.vector.tensor_scalar_max(h1Y[:], pY[:], 0.0)
                        h1s[g] = (h1X, h1Y)

                    if g < 1:
                        continue
                    gg = g - 1
                    h1X, h1Y = h1s.pop(gg)

                    # layer 2: 4 block-diagonal 4-node matmuls (128x48)
                    l2b = l2ps.tile([128, 384], FP32, tag="l2")
                    if sim_init:
                        nc.vector.memset(l2b[:], 0.0)
                    for yb in range(2):
                        src = h1X if yb == 0 else h1Y
                        for cb in range(2):
                            j = gg * 4 + yb * 2 + cb
                            nc.tensor.matmul(
                                l2b[64 * cb:64 * cb + 48,
                                    192 * yb:192 * yb + BT],
                                w2bd[:, j * 48:(j + 1) * 48],
                                src[:, cb * BT:(cb + 1) * BT],
                                tile_position=(0, 64 * cb),
                            )

                    # evacuate and DMA out, every 2 groups
                    if gg % 2 == 0:
                        otq = outp.tile([128, 768], FP32, tag="out")
                        nc.vector.tensor_copy(otq[:, 0:384], l2b[:])
                    else:
                        sg = gg // 2
                        nc.scalar.copy(otq[:, 384:768], l2b[:])
                        for hf in range(2):
                            eng = nc.sync if hf == 0 else nc.scalar
                            eng.dma_start(
                                out_ext[sg, hf], otq[64 * hf:64 * hf + 48, :])

    nc.finalize()
    if legalize:
        _legalize_waits(nc)
    return nc


def _legalize_waits(nc, keep_max=1, nop_max=1):
    """Hoist excess per-instruction semaphore waits onto same-engine NOPs.

    This walrus build rejects instructions carrying more than a couple of
    sync-wait commands ("Too many sync wait commands"). Tile attaches all
    required waits directly to consumer instructions; split them onto
    preceding InstNoOps on the same engine (semantically identical: the
    sequencer performs the waits in order before the real instruction).
    """
    ctr = [0]

    def mknop(engine, waits):
        ctr[0] += 1
        return mybir.InstNoOp(
            name=f"I-whoist-{ctr[0]}", engine=engine, bass_nofuse=True,
            sync_info=mybir.SyncInfo(on_wait=list(waits), on_update=[]))

    for f in nc.m.functions:
        for blk in f.blocks:
            out = []
            for inst in blk.instructions:
                si = getattr(inst, 'sync_info', None)
                eng = getattr(inst, 'engine', None)
                if si is not None and eng is not None and len(si.on_wait) > keep_max:
                    waits = list(si.on_wait)
                    keep, hoist = waits[:keep_max], waits[keep_max:]
                    for i in range(0, len(hoist), nop_max):
                        out.append(mknop(eng, hoist[i:i + nop_max]))
                    inst.sync_info = mybir.SyncInfo(
                        on_wait=keep, on_update=list(si.on_update))
                out.append(inst)
            blk.instructions = out


def _get_nc():
    global _cached_nc
    if _cached_nc is None:
        _cached_nc = _build_nc()
    return _cached_nc


def _prep_inputs(x, node_label, weights_pool1, weights_pool2):
    """Shard + pre-transpose full inputs into per-core in_maps."""
    x = np.ascontiguousarray(x, dtype=np.float32)
    node_label = np.ascontiguousarray(node_label, dtype=np.float32)
    p1 = np.ascontiguousarray(
        weights_pool1.transpose(0, 2, 1), dtype=np.float32).reshape(C, H * D)
    p2 = np.ascontiguousarray(
        weights_pool2.transpose(0, 2, 1), dtype=np.float32).reshape(C, O * H)

    # x -> [n, d, bt]
    x_t = np.ascontiguousarray(x.transpose(1, 3, 0, 2)).reshape(N, D, BT)

    # node m for (yb, cb, s) within a group
    m_arr = np.empty((2, 2, 4), dtype=np.int64)
    for yb in range(2):
        for cb in range(2):
            for s in range(4):
                m_arr[yb, cb, s] = _m_of(yb, cb, s)

    in_maps = []
    for k in range(NCORES):
        lab = node_label[k * NSH:(k + 1) * NSH]            # [256, 8]
        xs = x_t[k * NSH:(k + 1) * NSH]                    # [256, 64, 192]
        # x_dev[g, 64p+d, k8*192+bt] = x_t[16g + 2*k8 + p, d, bt]
        xdev = xs.reshape(NGROUPS, 8, 2, D, BT).transpose(0, 2, 3, 1, 4)
        xdev = xdev.reshape(NGROUPS, 128, 8 * BT)
        # pack 2 groups per DMA block
        xdev = np.ascontiguousarray(
            xdev.reshape(8, 2, 128, 8 * BT).transpose(0, 2, 1, 3)
        ).reshape(8, 128, 2 * 8 * BT)
        # label_w1[c, p*128+q] = lab[2q+p, c]
        lw1 = lab.reshape(NPAIR, 2, C).transpose(2, 1, 0).reshape(C, NSH)
        # label_w2[c, s*64 + g*4 + j_local] = lab[16g + m_arr[...], c]
        # j_local = yb*2 + cb
        gidx = np.empty((4, NGROUPS, 4), dtype=np.int64)
        for s in range(4):
            for g in range(NGROUPS):
                for jl in range(4):
                    yb, cb = jl // 2, jl % 2
                    gidx[s, g, jl] = 16 * g + m_arr[yb, cb, s]
        lw2 = lab[gidx.reshape(-1)].reshape(4, NGROUPS, 4, C) \
            .transpose(3, 0, 1, 2).reshape(C, NSH)
        wconst = np.ascontiguousarray(
            np.concatenate([p1, p2, lw1, lw2], axis=1))    # [8, 2944]
        in_maps.append({"x_dev": xdev, "wconst": wconst})
    return in_maps


def _unpack_outputs(results):
    """Per-core out_dev [sg, hf, 12s+o, gg*384+yb*192+bt] -> (B, N, T, O)."""
    out = np.empty((B, N, T, O), dtype=np.float32)
    m_arr = np.empty((2, 2, 4), dtype=np.int64)
    for yb in range(2):
        for cb in range(2):
            for s in range(4):
                m_arr[yb, cb, s] = _m_of(yb, cb, s)
    for k in range(NCORES):
        od = np.asarray(results[k]["out_dev"]).reshape(
            NGROUPS // 2, 2, 4, O, 2, 2, BT)   # [sg, hf(=cb), s, o, gg, yb, bt]
        od = od.transpose(0, 4, 5, 1, 2, 3, 6)  # [sg, gg, yb, cb, s, o, bt]
        # node local l = 16*(2*sg+gg) + m_arr[yb, cb, s]
        sg = np.arange(NGROUPS // 2)[:, None, None, None, None]
        gg = np.arange(2)[None, :, None, None, None]
        l_arr = 16 * (2 * sg + gg) + m_arr[None, None, :, :, :]
        out_core = np.empty((NSH, O, BT), dtype=np.float32)
        out_core[l_arr.reshape(-1)] = od.reshape(-1, O, BT)
        oc = out_core.reshape(NSH, O, B, T).transpose(2, 0, 3, 1)
        out[:, k * NSH:(k + 1) * NSH] = oc
    return out


def kernel(x, node_label, weights_pool1, weights_pool2):
    global last_exec_time_ns, last_results
    nc = _get_nc()
    in_maps = _prep_inputs(x, node_label, weights_pool1, weights_pool2)
    res = run_bass_kernel_spmd(nc, in_maps, core_ids=list(range(NCORES)))
    last_exec_time_ns = res.exec_time_ns
    last_results = res
    return _unpack_outputs(res.results)


# revision 43
# speedup vs baseline: 1.6333x; 1.0885x over previous
"""Trainium2 Bass kernel for the per-node adaptive output layer (gnn_message_passing).

Computation (per node n):
    w1[n] = sum_c label[n,c] * pool1[c]          (64x32)
    w2[n] = sum_c label[n,c] * pool2[c]          (32x12)
    h     = relu(x[:, n, :]) @ w1[n]             (192x64 @ 64x32)
    out   = relu(h) @ w2[n]                      (192x32 @ 32x12)

Distribution: shard N=2048 nodes across 8 NeuronCores (256 nodes/core),
weight pools replicated, labels sharded with N. No collectives.

Per-core schedule (256 nodes, 16 groups of 16 nodes = 8 even/odd pairs):
  - x arrives via SWDGE cast-DMA (fp32->bf16) in [128, 2*8*192] blocks:
      partition = 64*(m%2) + d, free col = (m//2)*192 + bt
  - Layer 1 packs an (even, odd) node pair into one K=128 matmul with a
    block-diagonal [128, 64] weight tile (8 MMs/group, 2-way column tiling).
  - Layer 2 packs FOUR nodes into one K=128 matmul with a 4x[32,12]
    block-diagonal weight tile (4 MMs/group); outputs land densely on
    48-partition spans, giving well-formed output DMAs.
  - Per-node weights are computed on device from the pools (K=8 matmuls),
    fp32 PSUM, written to bf16 block-diagonal stationary layouts.
"""

import sys
import types

import ml_dtypes
import numpy as np

import concourse.bass as bass
import concourse.mybir as mybir
from concourse import tile
from concourse.bass_utils import run_bass_kernel_spmd


def _ensure_ntff_hook():
    """Register the NTFF profiling hook if the image's antenv lacks it.

    bass_utils' axon trace path imports antenv.axon_hooks unconditionally
    when BASS_TRACE is set; provide it from trn_agent_boot when missing so
    tracing works instead of crashing. Best-effort only.
    """
    try:
        from antenv import axon_hooks  # noqa: F401
        return
    except ImportError:
        pass
    try:
        import antenv
        from trn_agent_boot.trn_boot import _ntff_profile_via_ctypes
        hook = [_ntff_profile_via_ctypes("/opt/axon/libaxon_pjrt.so")]
        mod = types.ModuleType("antenv.axon_hooks")
        mod.get_axon_ntff_profile_hook = lambda: hook[0]
        mod.set_axon_ntff_profile_hook = lambda h: hook.__setitem__(0, h)
        sys.modules["antenv.axon_hooks"] = mod
        antenv.axon_hooks = mod
    except Exception:
        pass


_ensure_ntff_hook()

# Problem shape (hardcoded per harness contract)
B, N, T, D = 16, 2048, 12, 64
C, H, O = 8, 32, 12
NCORES = 8
NSH = N // NCORES            # 256 nodes per core
BT = B * T                   # 192
NGROUPS = 16                 # node groups per core
GN = 16                      # nodes per group
NPAIR = NSH // 2             # 128 node pairs per core

FP32 = mybir.dt.float32
BF16 = mybir.dt.bfloat16
RELU = mybir.ActivationFunctionType.Relu

# Within a group, node index m (0..15): p = m%2 (L1 partition half),
# k8 = m//2 (pair index / x free-col block).
# Layer-2 regrouping: each L2 matmul j covers 4 nodes, one per slot
# s (0..3); slot s of matmul (yb, cb) is node k8 = 4*yb + 2*cb + s//2,
# p = s%2.  (yb = psum bank X/Y of layer 1, cb = col block within bank.)


def _m_of(yb, cb, s):
    k8 = 4 * yb + 2 * cb + (s // 2)
    return 2 * k8 + (s % 2)


last_exec_time_ns = None
last_results = None
_cached_nc = None


def _build_nc(legalize=True, sim_init=False):
    nc = bass.Bass()

    # x packed as 8 blocks of 2 groups: [sb2, 64p+d, g2*1536 + k8*192 + bt]
    x_ext = nc.declare_dram_parameter(
        "x_dev", [NGROUPS // 2, 128, 2 * 8 * BT], FP32, isOutput=False)
    # pools + labels merged (bf16, cast on host): pool1 (c,h,d) [0:2048] |
    # pool2 (c,o,k) [2048:2432] | label_w1 [2432:2688] | label_w2 [2688:2944]
    wc_ext = nc.declare_dram_parameter("wconst", [C, 2944], BF16, isOutput=False)
    # out: [sg, half, 12s+o, gg*384 + yb*192 + bt]
    out_ext = nc.declare_dram_parameter(
        "out_dev", [NGROUPS // 2, 2, 48, 2 * 2 * BT], FP32, isOutput=True)

    with tile.TileContext(nc) as tc:
        with tc.tile_pool(name="persist", bufs=1) as persist:
            # block-diagonal stationary weights, bf16. w1 is split into two
            # tensors (even / odd groups) so the hypernetwork evacuation
            # copies form two independent chains (Tile's byte-range overlap
            # tracking serializes all writers of one tensor).
            # w1bdX[64p+d, ql*64 + 32p + h] = w1[2q+p][d, h]; zeros elsewhere
            w1bdA = persist.tile([128, NPAIR // 2 * 2 * H], BF16)  # even g
            w1bdB = persist.tile([128, NPAIR // 2 * 2 * H], BF16)  # odd g
            # w2bd[32s+k, j*48 + 12s + o] = w2[node(j, s)][k, o]; zeros else
            w2bd = persist.tile([128, (NSH // 4) * 4 * O], BF16)
            wconst = persist.tile([C, 2944], BF16)

            nc.vector.memset(w1bdA[:], 0.0)
            nc.vector.memset(w1bdB[:], 0.0)
            nc.vector.memset(w2bd[:], 0.0)

            # consts already bf16: fast HWDGE load, no SWDGE queueing
            nc.sync.dma_start(wconst[:], wc_ext[:])
            pool1 = wconst[:, 0:2048]                # (c, h*64+d)
            pool2 = wconst[:, 2048:2432]             # (c, o*32+k)
            label1 = wconst[:, 2432:2688]            # cols p*128+q
            label2 = wconst[:, 2688:2944]            # cols s*64 + (g*4+j_local)

            # ---------- hypernetwork: per-node weights ----------
            with tc.tile_pool(name="wpsum", bufs=4, space="PSUM") as wpsum:
                # w1: out[d, q] = sum_c pool1[c,h,d]*label1[c,q], both parities
                for hc in range(H // 4):         # 8 chunks of 4 h values
                    wp = wpsum.tile([128, 512], FP32, tag="wp")
                    for h4 in range(4):
                        h = hc * 4 + h4
                        for p in range(2):
                            nc.tensor.matmul(
                                wp[64 * p:64 * p + 64, h4 * 128:(h4 + 1) * 128],
                                pool1[:, h * D:(h + 1) * D],            # [8, 64]
                                label1[:, p * NPAIR:(p + 1) * NPAIR],   # [8, 128]
                                tile_position=(0, 64 * p),
                            )
                    # psum[64p+d, (h4, g, k8)] -> w1bd{A,B}[64p+d,
                    #   (g//2 * 8 + k8)*64 + 32p + hc*4 + h4]
                    for p in range(2):
                        src2 = wp[64 * p:64 * p + 64, :].rearrange(
                            "pp (h ge par k) -> pp par ge k h",
                            h=4, ge=8, par=2, k=8)
                        for t, w1t in enumerate((w1bdA, w1bdB)):
                            src = src2[:, t]
                            dst = w1t[64 * p:64 * p + 64, :].rearrange(
                                "pp (ge k h) -> pp ge k h", ge=8, k=8)[
                                :, :, :,
                                32 * p + hc * 4:32 * p + hc * 4 + 4]
                            if t == 0:
                                nc.scalar.copy(dst, src)
                            else:
                                nc.vector.tensor_copy(dst, src)

                # w2: out[k, idx] = sum_c pool2[c,o,k]*label2[c, s*64+idx]
                for half in range(2):
                    wp2 = wpsum.tile([128, 384], FP32, tag="wp")
                    for o6 in range(6):
                        o = half * 6 + o6
                        for s in range(4):
                            nc.tensor.matmul(
                                wp2[32 * s:32 * s + 32, o6 * 64:(o6 + 1) * 64],
                                pool2[:, o * H:(o + 1) * H],            # [8, 32]
                                label2[:, s * 64:(s + 1) * 64],         # [8, 64]
                                tile_position=(0, 32 * s),
                            )
                    # psum[32s+k, (o6 idx)] -> w2bd[32s+k, idx*48 + 12s + o]
                    for s in range(4):
                        src = wp2[32 * s:32 * s + 32, :].rearrange(
                            "p (o i) -> p i o", o=6)
                        dst = w2bd[32 * s:32 * s + 32, :].rearrange(
                            "p (i o) -> p i o", o=4 * O)[
                            :, :, 12 * s + half * 6:12 * s + half * 6 + 6]
                        nc.vector.tensor_copy(dst, src)

            # ---------- main loop ----------
            with (
                tc.tile_pool(name="xin", bufs=8) as xin,
                tc.tile_pool(name="h1p", bufs=4) as h1p,
                tc.tile_pool(name="outp", bufs=4) as outp,
                tc.tile_pool(name="l1ps", bufs=4, space="PSUM") as l1ps,
                tc.tile_pool(name="l2ps", bufs=4, space="PSUM") as l2ps,
            ):
                otq = None
                xt2 = None
                h1s = {}
                # software pipeline, 1-group skew: L1(g) issues before L2(g-1)
                for g in range(NGROUPS + 1):
                    if g < NGROUPS:
                        if g % 2 == 0:
                            xt2 = xin.tile([128, 2 * 8 * BT], BF16, tag="x")
                            nc.gpsimd.dma_start(xt2[:], x_ext[g // 2])  # cast
                            nc.vector.tensor_scalar_max(
                                xt2[:], xt2[:], 0.0)  # relu(x)
                        xt = xt2[:, (g % 2) * 8 * BT:(g % 2 + 1) * 8 * BT]

                        # layer 1: 8 block-diagonal pair matmuls (128x64)
                        pX = l1ps.tile([128, 384], FP32, tag="l1")  # pairs 0-3
                        pY = l1ps.tile([128, 384], FP32, tag="l1")  # pairs 4-7
                        for k8 in range(8):
                            q = g * 8 + k8
                            dst = pX if k8 < 4 else pY
                            cb = (k8 % 4) // 2
                            w1t = w1bdA if g % 2 == 0 else w1bdB
                            ql = (g // 2) * 8 + k8
                            nc.tensor.matmul(
                                dst[64 * (k8 % 2):64 * (k8 % 2) + 64,
                                    # pairs (0,1)|(2,3) share a col range
                                    192 * cb:192 * cb + BT],
                                w1t[:, ql * 64:(ql + 1) * 64],
                                xt[:, k8 * BT:(k8 + 1) * BT],
                                tile_position=(0, 64 * (k8 % 2)),
                            )

                        # relu + cast to bf16, psum -> sbuf
                        h1X = h1p.tile([128, 384], BF16, tag="h1")
                        h1Y = h1p.tile([128, 384], BF16, tag="h1")
                        nc.scalar.activation(h1X[:], pX[:], RELU)
                        nc.scalar.activation(h1Y[:], pY[:], RELU)
                        h1s[g] = (h1X, h1Y)

                    if g < 1:
                        continue
                    gg = g - 1
                    h1X, h1Y = h1s.pop(gg)

                    # layer 2: 4 block-diagonal 4-node matmuls (128x48)
                    l2b = l2ps.tile([128, 384], FP32, tag="l2")
                    if sim_init:
                        nc.vector.memset(l2b[:], 0.0)
                    for yb in range(2):
                        src = h1X if yb == 0 else h1Y
                        for cb in range(2):
                            j = gg * 4 + yb * 2 + cb
                            nc.tensor.matmul(
                                l2b[64 * cb:64 * cb + 48,
                                    192 * yb:192 * yb + BT],
                                w2bd[:, j * 48:(j + 1) * 48],
                                src[:, cb * BT:(cb + 1) * BT],
                                tile_position=(0, 64 * cb),
                            )

                    # evacuate and DMA out, every 2 groups
                    if gg % 2 == 0:
                        otq = outp.tile([128, 768], FP32, tag="out")
                        nc.vector.tensor_copy(otq[:, 0:384], l2b[:])
                    else:
                        sg = gg // 2
                        nc.vector.tensor_copy(otq[:, 384:768], l2b[:])
                        for hf in range(2):
                            nc.sync.dma_start(
                                out_ext[sg, hf], otq[64 * hf:64 * hf + 48, :])

    nc.finalize()
    if legalize:
        _legalize_waits(nc)
    return nc


def _legalize_waits(nc, keep_max=1, nop_max=1):
    """Hoist excess per-instruction semaphore waits onto same-engine NOPs.

    This walrus build rejects instructions carrying more than a couple of
    sync-wait commands ("Too many sync wait commands"). Tile attaches all
    required waits directly to consumer instructions; split them onto
    preceding InstNoOps on the same engine (semantically identical: the
    sequencer performs the waits in order before the real instruction).
    """
    ctr = [0]

    def mknop(engine, waits):
        ctr[0] += 1
        return mybir.InstNoOp(
            name=f"I-whoist-{ctr[0]}", engine=engine, bass_nofuse=True,
            sync_info=mybir.SyncInfo(on_wait=list(waits), on_update=[]))

    for f in nc.m.functions:
        for blk in f.blocks:
            out = []
            for inst in blk.instructions:
                si = getattr(inst, 'sync_info', None)
                eng = getattr(inst, 'engine', None)
                if si is not None and eng is not None and len(si.on_wait) > keep_max:
                    waits = list(si.on_wait)
                    keep, hoist = waits[:keep_max], waits[keep_max:]
                    for i in range(0, len(hoist), nop_max):
                        out.append(mknop(eng, hoist[i:i + nop_max]))
                    inst.sync_info = mybir.SyncInfo(
                        on_wait=keep, on_update=list(si.on_update))
                out.append(inst)
            blk.instructions = out


def _get_nc():
    global _cached_nc
    if _cached_nc is None:
        _cached_nc = _build_nc()
    return _cached_nc


def _prep_inputs(x, node_label, weights_pool1, weights_pool2):
    """Shard + pre-transpose full inputs into per-core in_maps."""
    x = np.ascontiguousarray(x, dtype=np.float32)
    node_label = np.ascontiguousarray(node_label, dtype=np.float32)
    p1 = np.ascontiguousarray(
        weights_pool1.transpose(0, 2, 1), dtype=np.float32).reshape(C, H * D)
    p2 = np.ascontiguousarray(
        weights_pool2.transpose(0, 2, 1), dtype=np.float32).reshape(C, O * H)

    # x -> [n, d, bt]
    x_t = np.ascontiguousarray(x.transpose(1, 3, 0, 2)).reshape(N, D, BT)

    # node m for (yb, cb, s) within a group
    m_arr = np.empty((2, 2, 4), dtype=np.int64)
    for yb in range(2):
        for cb in range(2):
            for s in range(4):
                m_arr[yb, cb, s] = _m_of(yb, cb, s)

    in_maps = []
    for k in range(NCORES):
        lab = node_label[k * NSH:(k + 1) * NSH]            # [256, 8]
        xs = x_t[k * NSH:(k + 1) * NSH]                    # [256, 64, 192]
        # x_dev[g, 64p+d, k8*192+bt] = x_t[16g + 2*k8 + p, d, bt]
        xdev = xs.reshape(NGROUPS, 8, 2, D, BT).transpose(0, 2, 3, 1, 4)
        xdev = xdev.reshape(NGROUPS, 128, 8 * BT)
        # pack 2 groups per DMA block
        xdev = np.ascontiguousarray(
            xdev.reshape(8, 2, 128, 8 * BT).transpose(0, 2, 1, 3)
        ).reshape(8, 128, 2 * 8 * BT)
        # label_w1[c, p*128+q] = lab[2q+p, c]
        lw1 = lab.reshape(NPAIR, 2, C).transpose(2, 1, 0).reshape(C, NSH)
        # label_w2[c, s*64 + g*4 + j_local] = lab[16g + m_arr[...], c]
        # j_local = yb*2 + cb
        gidx = np.empty((4, NGROUPS, 4), dtype=np.int64)
        for s in range(4):
            for g in range(NGROUPS):
                for jl in range(4):
                    yb, cb = jl // 2, jl % 2
                    gidx[s, g, jl] = 16 * g + m_arr[yb, cb, s]
        lw2 = lab[gidx.reshape(-1)].reshape(4, NGROUPS, 4, C) \
            .transpose(3, 0, 1, 2).reshape(C, NSH)
        wconst = np.ascontiguousarray(
            np.concatenate([p1, p2, lw1, lw2], axis=1)).astype(
            ml_dtypes.bfloat16)                            # [8, 2944]
        in_maps.append({"x_dev": xdev, "wconst": wconst})
    return in_maps


def _unpack_outputs(results):
    """Per-core out_dev [sg, hf, 12s+o, gg*384+yb*192+bt] -> (B, N, T, O)."""
    out = np.empty((B, N, T, O), dtype=np.float32)
    m_arr = np.empty((2, 2, 4), dtype=np.int64)
    for yb in range(2):
        for cb in range(2):
            for s in range(4):
                m_arr[yb, cb, s] = _m_of(yb, cb, s)
    for k in range(NCORES):
        od = np.asarray(results[k]["out_dev"]).reshape(
            NGROUPS // 2, 2, 4, O, 2, 2, BT)   # [sg, hf(=cb), s, o, gg, yb, bt]
        od = od.transpose(0, 4, 5, 1, 2, 3, 6)  # [sg, gg, yb, cb, s, o, bt]
        # node local l = 16*(2*sg+gg) + m_arr[yb, cb, s]
        sg = np.arange(NGROUPS // 2)[:, None, None, None, None]
        gg = np.arange(2)[None, :, None, None, None]
        l_arr = 16 * (2 * sg + gg) + m_arr[None, None, :, :, :]
        out_core = np.empty((NSH, O, BT), dtype=np.float32)
        out_core[l_arr.reshape(-1)] = od.reshape(-1, O, BT)
        oc = out_core.reshape(NSH, O, B, T).transpose(2, 0, 3, 1)
        out[:, k * NSH:(k + 1) * NSH] = oc
    return out


def kernel(x, node_label, weights_pool1, weights_pool2):
    global last_exec_time_ns, last_results
    nc = _get_nc()
    in_maps = _prep_inputs(x, node_label, weights_pool1, weights_pool2)
    res = run_bass_kernel_spmd(nc, in_maps, core_ids=list(range(NCORES)))
    last_exec_time_ns = res.exec_time_ns
    last_results = res
    return _unpack_outputs(res.results)
